# revision 85
# baseline (speedup 1.0000x reference)
"""Trainium2 Bass kernel for nn_AttentionBlock (dense transformer block).

Data-parallel over batch: each of the 8 NeuronCores processes one batch
element end-to-end (no collectives). Activations are channel-major
(C on partitions, tokens on free). Large matmuls in bf16 with fp32 PSUM
accumulation. Partition reductions (layernorm stats, softmax denominators)
via ones-vector matmuls; partition broadcasts via K=1 ones-row matmuls.

LayerNorm scale/bias are folded into the downstream projection weights on
the host; on-device LN computes xhat = x*A + B with A=1/std, B=-mu/std
broadcast via rank-1 matmuls, evacuated to SBUF bf16 by the scalar engine,
and applied with 2x-mode bf16 vector ops split across DVE and GpSimd.
Elementwise work is spread across DVE / Activation / GpSimd so the PE
stays fed.
"""
import math
import numpy as np
from contextlib import ExitStack

import concourse.bass as bass
import concourse.bacc as bacc
import concourse.mybir as mybir
import concourse.tile as tile

P = 128
C = 640
CT = C // P          # 5
HW = 1024
NHALF = 2
NH = 8               # heads
DH = 80              # head dim
GROUPS = 32
GSIZE = C // GROUPS  # 20
DCTX = 512
LCTX = 77
LCTXP = 80           # padded context length
FFN = 5120
FFH = 2560
FT = FFH // P        # 20

F32 = mybir.dt.float32
F32R = mybir.dt.bfloat16
F8 = mybir.dt.float8e4
DR = mybir.MatmulPerfMode.DoubleRow
W8SCALE = 32.0       # fp8 weight pre-scale (keeps 0.02-scale weights normal)
AF = mybir.ActivationFunctionType
ALU = mybir.AluOpType
AX = mybir.AxisListType
SCALE = 1.0 / math.sqrt(DH)
NHP = 112            # per-head column stride in vp (80 vals + ones@96; 16B-aligned)
DHB = DH + 1         # out-proj weight rows per head (row 80 carries the bias)

_CACHE = {}


def _pcs(dram_ap):
    return dram_ap.rearrange("(t p) -> p t", p=P)


def _build(stages=5, reps=1, debug=None):
    nc = bacc.Bacc("TRN2", target_bir_lowering=False, debug=False)

    xt_d = nc.dram_tensor("xt", [C, HW], F32, kind="ExternalInput")
    xtb_d = nc.dram_tensor("xtb", [C, HW], F32R, kind="ExternalInput")
    ctxT_d = nc.dram_tensor("ctxT", [DCTX, LCTXP], F32R, kind="ExternalInput")

    def w_in(name, shape):
        return nc.dram_tensor(name, shape, F32R, kind="ExternalInput")

    def w8_in(name, shape):
        return nc.dram_tensor(name, shape, F8, kind="ExternalInput")

    conv1_wT = w_in("conv1_wT", [C, C])
    sa_in_w = w8_in("sa_in_w", [C, 3 * C])     # ln1_s, q-SCALE, x32 folded on host
    sa_out_w = w8_in("sa_out_w", [DHB * NH, C])  # row 80 of head 0 = bias*32
    ca_q_w = w8_in("ca_q_w", [C, C])           # ln2_s, SCALE, x32 folded on host
    ca_k_w = w_in("ca_k_w", [DCTX, C])
    ca_v_w = w_in("ca_v_w", [DCTX, C])
    ca_out_w = w8_in("ca_out_w", [DHB * NH, C])
    lin1_w = w8_in("lin1_w", [C, FFN])
    lin2_w = w8_in("lin2_w", [FFH, C])
    co_wT = w_in("co_wT", [C, C])
    G_d = w_in("G", [C, GROUPS])
    G2_d = w_in("G2", [GROUPS, C])
    recipC_d = w_in("recipC", [P, 1])          # 1/C column for LN stats
    vpinit_d = w8_in("vpinit", [P, NH * NH * NHP])
    ohrow_d = w8_in("ohrow", [1, NH * HW])
    vcainit_d = w_in("vcainit", [LCTXP, NH * 97])
    qkb_d = nc.dram_tensor("qkb", [DH, 2 * NH], F32, kind="ExternalInput")
    caqb_d = nc.dram_tensor("caqb", [DH, NH], F32, kind="ExternalInput")

    vecs = {}
    for name in ["gn_s", "gn_b", "conv1_b", "co_b"]:
        vecs[name] = nc.dram_tensor(name, [C], F32, kind="ExternalInput")
    lin1_b_d = nc.dram_tensor("lin1_b", [FFN], F32, kind="ExternalInput")

    y_d = nc.dram_tensor("y", [C, HW], F32, kind="ExternalOutput")

    with tile.TileContext(nc) as tc, ExitStack() as top:
        cpool = top.enter_context(tc.tile_pool(name="consts", bufs=1))
        respool = top.enter_context(tc.tile_pool(name="resid", bufs=1))
        tbpool = top.enter_context(tc.tile_pool(name="tbigs", bufs=1))

        nvec = len(vecs)
        vpack = cpool.tile([P, nvec * CT + FFN // P + 2], F32, tag="vpack")
        vt = {}
        for i, (name, d) in enumerate(vecs.items()):
            sl = vpack[:, i * CT:(i + 1) * CT]
            nc.sync.dma_start(sl, _pcs(d.ap()))
            vt[name] = sl
        lin1_b_sb = vpack[:, nvec * CT:nvec * CT + FFN // P]
        nc.sync.dma_start(lin1_b_sb, _pcs(lin1_b_d.ap()))
        epsln = vpack[:, nvec * CT + FFN // P:nvec * CT + FFN // P + 1]
        nc.gpsimd.memset(epsln, 1e-5)
        epsgn = vpack[:, nvec * CT + FFN // P + 1:nvec * CT + FFN // P + 2]
        nc.gpsimd.memset(epsgn, 1e-6)
        recipC_sb = cpool.tile([P, 1], F32R, tag="recipC")
        nc.sync.dma_start(recipC_sb[:], recipC_d.ap())
        G_sb = cpool.tile([P, CT, GROUPS], F32R, tag="G")
        nc.sync.dma_start(G_sb[:], G_d.ap().rearrange("(t p) g -> p t g", p=P))
        G2_sb = cpool.tile([GROUPS, C], F32R, tag="G2")
        nc.sync.dma_start(G2_sb[:], G2_d.ap())
        qkb_sb = cpool.tile([DH, 2 * NH], F32, tag="qkb")
        nc.sync.dma_start(qkb_sb[:], qkb_d.ap())
        caqb_sb = cpool.tile([DH, NH], F32, tag="caqb")
        nc.sync.dma_start(caqb_sb[:], caqb_d.ap())

        # ---------------- helpers ----------------
        def load_act_table(set_id, after=None):
            """Pre-place an act-table load so the greedy insert_act_table_loads
            pass sees Ln/Exp/Square/Copy (set 6) resident and doesn't thrash
            between per-function tables on every LayerNorm. `after` anchors
            the load in the schedule (dep-free instructions float to the
            top, which would leave the wrong table resident)."""
            ins = [nc.scalar.lower_ap(after)] if after is not None else []
            raw = mybir.InstLoadActFuncSet(
                name=nc.scalar.bass.get_next_instruction_name(),
                ins=ins, outs=[], act_func_set_id=set_id)
            return nc.scalar.add_instruction(raw)

        def layer_norm(phase_ctx, src, tag, eps_ap, big_out=None, pre=None):
            """src: CT [P,HW] bf16 tiles -> CT tiles, (x-mu)/std.
            ln scale/bias are folded into downstream weights on the host.
            big_out: dtype -> write one [P,CT,HW] tile (for DoubleRow rhs).
            pre(n): emits the producer's token-half-n work right before this
            half's stats, so the prior half's scalar chain overlaps it."""
            if big_out is not None:
                big = tbpool.tile([P, CT, HW], big_out, tag=f"tbig_{tag}",
                                  name=f"t_{tag}")
                out = [big[:, k] for k in range(CT)]
            else:
                big = None
                out = [tbpool.tile([P, HW], F32R, tag=f"t_{tag}{k}",
                                   name=f"t_{tag}{k}") for k in range(CT)]
            with ExitStack() as ctx:
                pool = ctx.enter_context(tc.tile_pool(name=f"ln_{tag}", bufs=2))
                ps = ctx.enter_context(tc.tile_pool(name=f"lnps_{tag}", bufs=1, space="PSUM"))
                sq = [pool.tile([P, HW], F32R, tag=f"sq{k}", name=f"sq{k}")
                      for k in range(CT)]
                mu_ps = ps.tile([1, HW], F32, tag="sx", name="mu_ps")
                m2_ps = ps.tile([1, HW], F32, tag="sxx", name="m2_ps")
                # everything is emitted per token-half so n0's scalar chain
                # (ACT/DVE/Pool) pipelines under n1's stats matmuls (PE).
                # mu and m2 chains share one stationary recipC column, so
                # the PE loads weights once for all 20 matmuls.
                for n in range(NHALF):
                    nsl = slice(n * 512, (n + 1) * 512)
                    if pre is not None:
                        pre(n)
                    for k in range(CT):
                        nc.vector.tensor_mul(sq[k][:, nsl], src[k][:, nsl],
                                             src[k][:, nsl])
                    for k in range(CT):
                        nc.tensor.matmul(mu_ps[:, nsl], lhsT=recipC_sb[:],
                                         rhs=src[k][:, nsl],
                                         start=(k == 0), stop=(k == CT - 1))
                    for k in range(CT):
                        nc.tensor.matmul(m2_ps[:, nsl], lhsT=recipC_sb[:],
                                         rhs=sq[k][:, nsl],
                                         start=(k == 0), stop=(k == CT - 1))
                    mu2 = pool.tile([1, 512], F32, tag="mu2", name="mu2")
                    nc.scalar.activation(mu2[:], mu_ps[:, nsl], AF.Square)
                    var = pool.tile([1, 512], F32, tag="var", name="var")
                    nc.vector.tensor_sub(var[:], m2_ps[:, nsl], mu2[:])
                    # 1/std = exp(-0.5*ln(var+eps)) — ln+exp live in ONE act
                    # table (with square/copy), so no table swap per LN.
                    lnv = pool.tile([1, 512], F32, tag="sd", name="lnv")
                    nc.scalar.activation(lnv[:], var[:], AF.Ln, bias=eps_ap[0:1])
                    A_row = pool.tile([1, 512], F32R, tag="Arow", name="A_row")
                    nc.scalar.activation(A_row[:], lnv[:], AF.Exp, scale=-0.5)
                    B_row = pool.tile([1, 512], F32R, tag="Brow", name="B_row")
                    with nc.allow_low_precision(reason="bf16 LN scale rows"):
                        nc.vector.scalar_tensor_tensor(B_row[:], mu_ps[:, nsl],
                                                       -1.0, A_row[:],
                                                       ALU.mult, ALU.mult)
                    # token-row scale/offset broadcast across partitions on
                    # the (otherwise idle) Pool engine
                    A_bc = pool.tile([P, 512], F32R, tag="Abc", name="A_bc")
                    nc.gpsimd.partition_broadcast(A_bc[:], A_row[:], channels=P)
                    B_bc = pool.tile([P, 512], F32R, tag="Bbc", name="B_bc")
                    nc.gpsimd.partition_broadcast(B_bc[:], B_row[:], channels=P)
                    for k in range(CT):
                        xa = pool.tile([P, 512], F32R, tag="xa", name="xa")
                        nc.vector.tensor_mul(xa[:], src[k][:, nsl], A_bc[:])
                        nc.vector.tensor_add(out[k][:, nsl], xa[:], B_bc[:])
            return (out, big) if big_out is not None else out

        def load_w(ctx, w_dram, kt, cout, tag, dt=F32R):
            wpool = ctx.enter_context(tc.tile_pool(name=f"w_{tag}", bufs=1))
            w_ap = w_dram.ap().rearrange("(t p) n -> p t n", p=P)
            wt = wpool.tile([P, kt, cout], dt, tag="w", name=f"w_{tag}")
            nc.sync.dma_start(wt[:], w_ap)
            return wt

        def linear_cm(ctx, wt, src, cout, tag, consumer):
            kt = len(src)
            mt_all = cout // P
            ps = ctx.enter_context(tc.tile_pool(name=f"ps_{tag}", bufs=4, space="PSUM"))
            for n in range(NHALF):
                for m in range(mt_all):
                    pst = ps.tile([P, 512], F32, tag="ps", name=f"ps_{tag}")
                    for k in range(kt):
                        nc.tensor.matmul(
                            pst[:], lhsT=wt[:, k, m * P:(m + 1) * P],
                            rhs=src[k][:, n * 512:(n + 1) * 512],
                            start=(k == 0), stop=(k == kt - 1))
                    consumer(m, n, pst)

        def linear_dr(ctx, wt, srcbig, cout, tag, consumer):
            """fp8 DoubleRow linear over K=640 (2 paired k-tiles + 1 plain)."""
            mt_all = cout // P
            ps = ctx.enter_context(tc.tile_pool(name=f"ps_{tag}", bufs=4, space="PSUM"))
            for n in range(NHALF):
                nsl = slice(n * 512, (n + 1) * 512)
                for m in range(mt_all):
                    msl = slice(m * P, (m + 1) * P)
                    pst = ps.tile([P, 512], F32, tag="ps", name=f"ps_{tag}")
                    nc.tensor.matmul(pst[:], lhsT=wt[:, 0:2, msl],
                                     rhs=srcbig[:, 0:2, nsl], perf_mode=DR,
                                     start=True, stop=False)
                    nc.tensor.matmul(pst[:], lhsT=wt[:, 2:4, msl],
                                     rhs=srcbig[:, 2:4, nsl], perf_mode=DR,
                                     start=False, stop=False)
                    nc.tensor.matmul(pst[:], lhsT=wt[:, 4, msl],
                                     rhs=srcbig[:, 4, nsl],
                                     start=False, stop=True)
                    consumer(m, n, pst)

        capre = top.enter_context(tc.tile_pool(name="capre", bufs=1))

        # ================= Phase 1: GroupNorm + conv1 =================
        for _rep in range(reps):
            x1 = [respool.tile([P, HW], F32R, tag=f"ra{k}", name=f"x1_{k}") for k in range(CT)]
            with ExitStack() as ctx:
                xopool = ctx.enter_context(tc.tile_pool(name="xop", bufs=1))
                x_orig = [xopool.tile([P, HW], F32R, tag=f"xo{k}", name=f"xo{k}") for k in range(CT)]
                # critical-path x loads go FIRST on the serial DMA queue
                # (bf16 copy: halves the critical-path DMA; the f32 x is
                # re-loaded in phase 4 for the long residual)
                for k in range(CT):
                    nc.sync.dma_start(x_orig[k][:], xtb_d.ap()[k * P:(k + 1) * P, :])
                conv1_sb = load_w(ctx, conv1_wT, CT, C, "conv1")

                # CA context K/V projections depend only on `context`; compute
                # them here where the PE is idle (x DMA / GN stats window).
                ctx_sb = capre.tile([P, 4, LCTXP], F32R, tag="ctx")
                nc.sync.dma_start(ctx_sb[:], ctxT_d.ap().rearrange("(t p) n -> p t n", p=P))
                wkpre = capre.tile([P, 4, C], F32R, tag="cawk", name="wk_ca")
                nc.sync.dma_start(wkpre[:], ca_k_w.ap().rearrange("(t p) n -> p t n", p=P))
                wvpre = capre.tile([P, 4, C], F32R, tag="cawv", name="wv_ca")
                nc.sync.dma_start(wvpre[:], ca_v_w.ap().rearrange("(t p) n -> p t n", p=P))
                vca = capre.tile([LCTXP, NH * 97], F32R, tag="vca")
                nc.sync.dma_start(vca[:], vcainit_d.ap())
                kca = capre.tile([DH, NH, LCTXP], F32R, tag="kca")
                with ExitStack() as kvctx:
                    kvps = kvctx.enter_context(tc.tile_pool(name="kvpre", bufs=2, space="PSUM"))
                    for h in range(NH):
                        kps = kvps.tile([DH, LCTXP], F32, tag="kvps", name="kps_ca")
                        for k in range(4):
                            nc.tensor.matmul(kps[:], lhsT=wkpre[:, k, h * DH:(h + 1) * DH],
                                             rhs=ctx_sb[:, k, :], start=(k == 0), stop=(k == 3))
                        nc.vector.tensor_scalar_mul(kca[:, h, :], kps[:], 1.0)
                    for nb in range(2):
                        vps = kvps.tile([LCTXP, 320], F32, tag="kvps", name="vps_ca")
                        for k in range(4):
                            nc.tensor.matmul(vps[:], lhsT=ctx_sb[:, k, :],
                                             rhs=wvpre[:, k, nb * 320:(nb + 1) * 320],
                                             start=(k == 0), stop=(k == 3))
                        nc.vector.tensor_scalar_mul(
                            vca[:].rearrange("p (h d) -> p h d", h=NH)
                            [:, nb * 4:(nb + 1) * 4, 0:DH],
                            vps[:].rearrange("p (h d) -> p h d", h=4), 1.0)
                pool = ctx.enter_context(tc.tile_pool(name="gn", bufs=2))
                sqpool = ctx.enter_context(tc.tile_pool(name="gnsq", bufs=2))
                t0pool = ctx.enter_context(tc.tile_pool(name="t0p", bufs=1))
                with ExitStack() as gctx:
                    gps_pool = gctx.enter_context(tc.tile_pool(name="gnps", bufs=1, space="PSUM"))
                    scs = pool.tile([P, CT, 2], F32, tag="scs")
                    for k in range(CT):
                        # sum(x) on DVE, sum(x^2) on ACT (Square + accumulator)
                        nc.vector.reduce_sum(scs[:, k, 0:1], x_orig[k][:], AX.X)
                        sqsc = sqpool.tile([P, HW], F32R, tag="sqsc", name="sqsc")
                        nc.scalar.activation(sqsc[:], x_orig[k][:], AF.Square,
                                             accum_out=scs[:, k, 1:2])
                        if k == 0:
                            # anchor the ln+exp table load after phase-1 data
                            # exists (dep via a dummy read) so it lands before
                            # GN's Ln instead of floating to the kernel top
                            load_act_table(6, after=sqsc[0:1, 0:1])
                    scs_r = pool.tile([P, CT, 2], F32R, tag="scsr")
                    nc.vector.tensor_scalar_mul(scs_r[:], scs[:], 1.0)
                    gps = gps_pool.tile([GROUPS, 2], F32, tag="g")
                    for k in range(CT):
                        nc.tensor.matmul(gps[:], lhsT=G_sb[:, k], rhs=scs_r[:, k],
                                         start=(k == 0), stop=(k == CT - 1))
                    NG = float(GSIZE * HW)
                    gmu = pool.tile([GROUPS, 1], F32, tag="gmu")
                    nc.vector.tensor_scalar_mul(gmu[:], gps[:, 0:1], 1.0 / NG)
                    gm2 = pool.tile([GROUPS, 1], F32, tag="gm2")
                    nc.vector.tensor_scalar_mul(gm2[:], gps[:, 1:2], 1.0 / NG)
                    gmu2 = pool.tile([GROUPS, 1], F32, tag="gmu2")
                    nc.vector.tensor_mul(gmu2[:], gmu[:], gmu[:])
                    gvar = pool.tile([GROUPS, 1], F32, tag="gvar")
                    nc.vector.tensor_sub(gvar[:], gm2[:], gmu2[:])
                    glnv = pool.tile([GROUPS, 1], F32, tag="gsd")
                    nc.scalar.activation(glnv[:], gvar[:], AF.Ln, bias=epsgn[:GROUPS])
                    gAB = pool.tile([GROUPS, 2], F32R, tag="gAB")
                    nc.scalar.activation(gAB[:, 0:1], glnv[:], AF.Exp, scale=-0.5)
                    with nc.allow_low_precision(reason="bf16 GN scale rows"):
                        nc.vector.scalar_tensor_tensor(gAB[:, 1:2], gmu[:], -1.0,
                                                       gAB[:, 0:1], ALU.mult, ALU.mult)
                    t0big = t0pool.tile([P, CT, HW], F32R, tag="t0", name="t0big")
                    for k in range(CT):
                        cps = gps_pool.tile([P, 2], F32, tag="cps")
                        nc.tensor.matmul(cps[:], lhsT=G2_sb[:, k * P:(k + 1) * P], rhs=gAB[:],
                                         start=True, stop=True)
                        cA = pool.tile([P, 1], F32, tag="cA", name="cA")
                        nc.vector.tensor_mul(cA[:], cps[:, 0:1], vt["gn_s"][:, k:k + 1])
                        cB = pool.tile([P, 1], F32, tag="cB", name="cB")
                        nc.vector.tensor_mul(cB[:], cps[:, 1:2], vt["gn_s"][:, k:k + 1])
                        nc.vector.tensor_add(cB[:], cB[:], vt["gn_b"][:, k:k + 1])
                        nc.vector.tensor_scalar(t0big[:, k], x_orig[k][:], cA[:], cB[:],
                                                ALU.mult, ALU.add)

                def conv1_consumer(m, n, pst):
                    # ACT evacuation: DVE is the busiest engine, ACT idles here
                    nsl = slice(n * 512, (n + 1) * 512)
                    nc.scalar.activation(x1[m][:, nsl], pst[:], AF.Identity,
                                         bias=vt["conv1_b"][:, m:m + 1])

                c1ps = ctx.enter_context(tc.tile_pool(name="ps_conv1", bufs=4,
                                                      space="PSUM"))

                def conv1_half(n):
                    nsl = slice(n * 512, (n + 1) * 512)
                    for m in range(CT):
                        pst = c1ps.tile([P, 512], F32, tag="ps", name="ps_conv1")
                        for k in range(CT):
                            nc.tensor.matmul(
                                pst[:], lhsT=conv1_sb[:, k, m * P:(m + 1) * P],
                                rhs=t0big[:, k, nsl],
                                start=(k == 0), stop=(k == CT - 1))
                        conv1_consumer(m, n, pst)

                for n in range(NHALF):
                    conv1_half(n)

            def _early_out(ctx_, tiles):
                ep = ctx_.enter_context(tc.tile_pool(name="early", bufs=2))
                for k in range(CT):
                    o = ep.tile([P, HW], F32, tag="eo", name="eo")
                    nc.vector.tensor_scalar_mul(o[:], tiles[k][:], 1.0)
                    nc.sync.dma_start(y_d.ap()[k * P:(k + 1) * P, :], o[:])

            if stages < 2:
                with ExitStack() as ectx:
                    _early_out(ectx, x1)
                continue

            # ================= Phase 2: LN1 + self-attention =================
            x2 = [respool.tile([P, HW], F32R, tag=f"rb{k}", name=f"x2_{k}") for k in range(CT)]
            with ExitStack() as ctx:
                wqkp = ctx.enter_context(tc.tile_pool(name="wqkp", bufs=1))
                wv = ctx.enter_context(tc.tile_pool(name="savw", bufs=1))
                vpool = ctx.enter_context(tc.tile_pool(name="vp", bufs=1))
                qk_sb = ctx.enter_context(tc.tile_pool(name="qksb", bufs=3))
                expp = ctx.enter_context(tc.tile_pool(name="expp", bufs=2))
                ohp = ctx.enter_context(tc.tile_pool(name="ohp", bufs=1))
                recp = ctx.enter_context(tc.tile_pool(name="recp", bufs=4))

                sa_in_ap = sa_in_w.ap().rearrange("(t p) n -> p t n", p=P)
                # oh rows 0..79 = normalized per-head outputs; row 80 is the
                # bias carrier (1.0 on head 0) for the 81-row out-proj weights
                oh = ohp.tile([DHB, NH, HW], F8, tag="oh")
                nc.sync.dma_start(oh[DH:DHB, :, :], ohrow_d.ap())
                qt, kt_ = {}, {}

                # prefetch all SA weights before LN1 so the DMAs overlap compute
                wq_sb = wqkp.tile([P, CT, C], F8, tag="wq")
                nc.sync.dma_start(wq_sb[:], sa_in_ap[:, :, 0:C])
                wk_sb = wqkp.tile([P, CT, C], F8, tag="wk")
                nc.sync.dma_start(wk_sb[:], sa_in_ap[:, :, C:2 * C])
                wv_sb = wv.tile([P, CT, C], F8, tag="wv", name="wv_sb")
                nc.sync.dma_start(wv_sb[:], sa_in_ap[:, :, 2 * C:3 * C])
                wo_sb = wv.tile([DHB, NH, C], F8, tag="wo", name="wo_sb")
                nc.sync.dma_start(wo_sb[:], sa_out_w.ap().rearrange("(h d) n -> d h n", d=DHB))
                # fp8 V, all 8 token-tiles in one tile so attn-out can DoubleRow
                vp = vpool.tile([P, NH, NH * NHP], F8, tag="vp", name="vp")
                nc.sync.dma_start(vp[:], vpinit_d.ap().rearrange(
                    "p (m c) -> p m c", m=NH))
                vp_h = vp[:].rearrange("p m (h c) -> p m h c", h=NH)

                t1, t1big = layer_norm(top, x1, "ln1", epsln, big_out=F8)

                with ExitStack() as actx:
                    ps_sc = actx.enter_context(tc.tile_pool(name="ps_sc", bufs=2, space="PSUM"))
                    ps_sqk = actx.enter_context(tc.tile_pool(name="ps_sqk", bufs=2, space="PSUM"))
                    ps_o = actx.enter_context(tc.tile_pool(name="ps_o", bufs=2, space="PSUM"))

                    for mk in range(NH):
                        for nb in range(2):
                            vps = ps_sqk.tile([P, 512], F32, tag="sps", name="vps")
                            nc.tensor.matmul(vps[:, :320], lhsT=t1big[:, 0:2, mk * P:(mk + 1) * P],
                                             rhs=wv_sb[:, 0:2, nb * 320:(nb + 1) * 320],
                                             perf_mode=DR, start=True, stop=False)
                            nc.tensor.matmul(vps[:, :320], lhsT=t1big[:, 2:4, mk * P:(mk + 1) * P],
                                             rhs=wv_sb[:, 2:4, nb * 320:(nb + 1) * 320],
                                             perf_mode=DR, start=False, stop=False)
                            nc.tensor.matmul(vps[:, :320], lhsT=t1big[:, 4, mk * P:(mk + 1) * P],
                                             rhs=wv_sb[:, 4, nb * 320:(nb + 1) * 320],
                                             start=False, stop=True)
                            nc.vector.tensor_scalar_mul(
                                vp_h[:, mk, nb * 4:(nb + 1) * 4, 0:DH],
                                vps[:, :320].rearrange("p (h d) -> p h d", h=4),
                                1.0 / W8SCALE)

                    def project_qk(h):
                        hsl = slice(h * DH, (h + 1) * DH)
                        q = qk_sb.tile([DH, HW], F32R, tag="qt", name="qtile")
                        kk = qk_sb.tile([DH, HW], F32R, tag="kt", name="ktile")
                        for n in range(NHALF):
                            nsl = slice(n * 512, (n + 1) * 512)
                            for (w_, out_sb, bcol) in ((wq_sb, q, h), (wk_sb, kk, NH + h)):
                                p_ = ps_sqk.tile([P, 512], F32, tag="sps", name="qkps")
                                nc.tensor.matmul(p_[:DH, :], lhsT=w_[:, 0:2, hsl],
                                                 rhs=t1big[:, 0:2, nsl], perf_mode=DR,
                                                 start=True, stop=False)
                                nc.tensor.matmul(p_[:DH, :], lhsT=w_[:, 2:4, hsl],
                                                 rhs=t1big[:, 2:4, nsl], perf_mode=DR,
                                                 start=False, stop=False)
                                nc.tensor.matmul(p_[:DH, :], lhsT=w_[:, 4, hsl],
                                                 rhs=t1big[:, 4, nsl],
                                                 start=False, stop=True)
                                nc.vector.tensor_scalar(out_sb[:, nsl], p_[:DH, :],
                                                        1.0 / W8SCALE,
                                                        qkb_sb[:, bcol:bcol + 1],
                                                        ALU.mult, ALU.add)
                        qt[h], kt_[h] = q, kk

                    def scores_exp(h):
                        e = expp.tile([P, NH, HW], F8, tag="exps", name="exps")
                        for mk in range(NH):
                            sps = ps_sc.tile([P, HW], F32, tag="sc", name="sps")
                            for n in range(NHALF):
                                nsl = slice(n * 512, (n + 1) * 512)
                                nc.tensor.matmul(sps[:, nsl],
                                                 lhsT=kt_[h][:, mk * P:(mk + 1) * P],
                                                 rhs=qt[h][:, nsl], start=True, stop=True)
                            nc.scalar.activation(e[:, mk, :], sps[:], AF.Exp)
                        return e

                    def attnout(h, e):
                        ops_n, rb_n, recp_dbg = [], [], []
                        for n in range(NHALF):
                            nsl = slice(n * 512, (n + 1) * 512)
                            ops_ = ps_o.tile([97, 512], F32, tag="ops")
                            for jp in range(NH // 2):
                                nc.tensor.matmul(ops_[:],
                                                 lhsT=vp[:, 2 * jp:2 * jp + 2,
                                                         h * NHP:h * NHP + 97],
                                                 rhs=e[:, 2 * jp:2 * jp + 2, nsl],
                                                 perf_mode=DR,
                                                 start=(jp == 0), stop=(jp == NH // 2 - 1))
                            rec = recp.tile([1, 512], F32, tag="rec", name="rec")
                            nc.vector.reciprocal(rec[:], ops_[96:97, :])
                            rb = recp.tile([DH, 512], F32, tag="rb", name="rb")
                            nc.gpsimd.partition_broadcast(rb[:], rec[:], channels=DH)
                            ops_n.append(ops_)
                            rb_n.append(rb)
                            recp_dbg.append(rec)
                        if debug == "att0" and h == 0:
                            dpool = actx.enter_context(tc.tile_pool(name="dbg", bufs=1))
                            for n in range(NHALF):
                                nsl = slice(n * 512, (n + 1) * 512)
                                d1 = dpool.tile([97, 512], F32, tag=f"dops{n}", name="dbg")
                                nc.vector.tensor_scalar_mul(d1[:], ops_n[n][:], 1.0)
                                nc.sync.dma_start(y_d.ap()[0:97, nsl], d1[:])
                                d2 = dpool.tile([DH, 512], F32, tag=f"drb{n}", name="dbg")
                                nc.vector.tensor_scalar_mul(d2[:], rb_n[n][:], 1.0)
                                nc.sync.dma_start(y_d.ap()[128:128 + DH, nsl], d2[:])
                                d3 = dpool.tile([1, 512], F32, tag=f"drec{n}", name="dbg")
                                nc.vector.tensor_scalar_mul(d3[:], recp_dbg[n][:], 1.0)
                                nc.sync.dma_start(y_d.ap()[100:101, nsl], d3[:])
                        for n in range(NHALF):
                            nsl = slice(n * 512, (n + 1) * 512)
                            nc.vector.tensor_mul(oh[:DH, h, nsl], ops_n[n][:DH, :],
                                                 rb_n[n][:])

                    project_qk(0)
                    e_cur = scores_exp(0)
                    if debug == "e0":
                        dpool = actx.enter_context(tc.tile_pool(name="dbg", bufs=1))
                        for mk in range(CT):
                            dt_ = dpool.tile([P, HW], F32, tag=f"d{mk}", name="dbg")
                            nc.vector.tensor_scalar_mul(dt_[:], e_cur[:, mk, :], 1.0)
                            nc.sync.dma_start(y_d.ap()[mk * P:(mk + 1) * P, :], dt_[:])
                    for h in range(NH):
                        if h + 1 < NH:
                            project_qk(h + 1)
                            e_nxt = scores_exp(h + 1)
                        attnout(h, e_cur)
                        if h + 1 < NH:
                            e_cur = e_nxt
                    if debug == "qk":
                        dpool = actx.enter_context(tc.tile_pool(name="dbg", bufs=1))
                        for h in range(NH):
                            dt_ = dpool.tile([DH, HW], F32, tag=f"d{h}", name="dbg")
                            nc.vector.tensor_scalar_mul(dt_[:], qt[h][:], 1.0)
                            nc.sync.dma_start(y_d.ap()[h * DH:(h + 1) * DH, :], dt_[:])
                    if debug == "kk":
                        dpool = actx.enter_context(tc.tile_pool(name="dbg", bufs=1))
                        for h in range(NH):
                            dt_ = dpool.tile([DH, HW], F32, tag=f"d{h}", name="dbg")
                            nc.vector.tensor_scalar_mul(dt_[:], kt_[h][:], 1.0)
                            nc.sync.dma_start(y_d.ap()[h * DH:(h + 1) * DH, :], dt_[:])
                if debug == "oh":
                    with ExitStack() as dctx:
                        dpool = dctx.enter_context(tc.tile_pool(name="dbg", bufs=1))
                        for h in range(NH):
                            dt_ = dpool.tile([DH, HW], F32, tag=f"d{h}", name="dbg")
                            nc.vector.tensor_scalar_mul(dt_[:], oh[:DH, h, :], 1.0)
                            nc.sync.dma_start(y_d.ap()[h * DH:(h + 1) * DH, :], dt_[:])
                if debug:
                    continue

                with ExitStack() as octx:
                    ps_out = octx.enter_context(tc.tile_pool(name="ps_saout", bufs=4, space="PSUM"))

                    def sa_outproj_half(n):
                        nsl = slice(n * 512, (n + 1) * 512)
                        for m in range(CT):
                            msl = slice(m * P, (m + 1) * P)
                            pst = ps_out.tile([P, 512], F32, tag="po", name="po")
                            for hp in range(NH // 2):
                                nc.tensor.matmul(pst[:],
                                                 lhsT=wo_sb[:, 2 * hp:2 * hp + 2, msl],
                                                 rhs=oh[:, 2 * hp:2 * hp + 2, nsl],
                                                 perf_mode=DR,
                                                 start=(hp == 0), stop=(hp == NH // 2 - 1))
                            nc.vector.scalar_tensor_tensor(
                                x2[m][:, nsl], pst[:], 1.0 / W8SCALE,
                                x1[m][:, nsl], ALU.mult, ALU.add)

                    for n in range(NHALF):
                        sa_outproj_half(n)

            if stages < 3:
                with ExitStack() as ectx:
                    _early_out(ectx, x2)
                continue

            # ================= Phase 3: LN2 + cross-attention =================
            x3 = [respool.tile([P, HW], F32R, tag=f"ra{k}", name=f"x3_{k}") for k in range(CT)]
            with ExitStack() as ctx:
                capool = ctx.enter_context(tc.tile_pool(name="ca", bufs=1))
                caw = ctx.enter_context(tc.tile_pool(name="caw", bufs=1))
                wqcap = ctx.enter_context(tc.tile_pool(name="wqcap", bufs=1))
                qcap = ctx.enter_context(tc.tile_pool(name="qca", bufs=3))
                expca = ctx.enter_context(tc.tile_pool(name="expca", bufs=3))
                recp = ctx.enter_context(tc.tile_pool(name="carecp", bufs=4))

                ohca = capool.tile([DHB, NH, HW], F8, tag="ohca")
                nc.sync.dma_start(ohca[DH:DHB, :, :], ohrow_d.ap())
                qtc = {}

                # prefetch remaining CA weights before LN2 (K/V context
                # projections were computed at the top of the rep body)
                wqca_sb = wqcap.tile([P, CT, C], F8, tag="wqca")
                nc.sync.dma_start(wqca_sb[:], ca_q_w.ap().rearrange("(t p) n -> p t n", p=P))
                woca_sb = caw.tile([DHB, NH, C], F8, tag="cawo", name="wo_ca")
                nc.sync.dma_start(woca_sb[:], ca_out_w.ap().rearrange("(h d) n -> d h n", d=DHB))

                t2, t2big = layer_norm(top, x2, "ln2", epsln, big_out=F8)

                with ExitStack() as actx:
                    ps_ca = actx.enter_context(tc.tile_pool(name="ps_ca", bufs=2, space="PSUM"))
                    ps_oca = actx.enter_context(tc.tile_pool(name="ps_oca", bufs=4, space="PSUM"))

                    def project_q_ca(h):
                        qp = ps_ca.tile([DH, HW], F32, tag="caps", name="qps_ca")
                        hsl = slice(h * DH, (h + 1) * DH)
                        for n in range(NHALF):
                            nsl = slice(n * 512, (n + 1) * 512)
                            nc.tensor.matmul(qp[:, nsl], lhsT=wqca_sb[:, 0:2, hsl],
                                             rhs=t2big[:, 0:2, nsl], perf_mode=DR,
                                             start=True, stop=False)
                            nc.tensor.matmul(qp[:, nsl], lhsT=wqca_sb[:, 2:4, hsl],
                                             rhs=t2big[:, 2:4, nsl], perf_mode=DR,
                                             start=False, stop=False)
                            nc.tensor.matmul(qp[:, nsl], lhsT=wqca_sb[:, 4, hsl],
                                             rhs=t2big[:, 4, nsl],
                                             start=False, stop=True)
                        q = qcap.tile([DH, HW], F32R, tag="qtca", name="qtca")
                        nc.scalar.activation(q[:], qp[:], AF.Identity,
                                             bias=caqb_sb[:, h:h + 1],
                                             scale=1.0 / W8SCALE)
                        qtc[h] = q

                    project_q_ca(0)
                    for h in range(NH):
                        sps = ps_ca.tile([LCTXP, HW], F32, tag="caps", name="sps_ca")
                        for n in range(NHALF):
                            nsl = slice(n * 512, (n + 1) * 512)
                            nc.tensor.matmul(sps[:, nsl], lhsT=kca[:, h, :], rhs=qtc[h][:, nsl],
                                             start=True, stop=True)
                        e = expca.tile([LCTXP, HW], F32R, tag="expca", name="expca_t")
                        # per-half exps so each attnout half starts sooner
                        for n in range(NHALF):
                            nsl = slice(n * 512, (n + 1) * 512)
                            nc.scalar.activation(e[:, nsl], sps[:, nsl], AF.Exp)
                        if h + 1 < NH:
                            project_q_ca(h + 1)
                        for n in range(NHALF):
                            nsl = slice(n * 512, (n + 1) * 512)
                            ops_ = ps_oca.tile([97, 512], F32, tag="opsca")
                            nc.tensor.matmul(ops_[:], lhsT=vca[:, h * 97:(h + 1) * 97],
                                             rhs=e[:, nsl], start=True, stop=True)
                            rec = recp.tile([1, 512], F32, tag="recca", name="recca")
                            nc.vector.reciprocal(rec[:], ops_[96:97, :])
                            rb = recp.tile([DH, 512], F32, tag="rbca", name="rbca")
                            nc.gpsimd.partition_broadcast(rb[:], rec[:], channels=DH)
                            nc.vector.tensor_mul(ohca[:DH, h, nsl], ops_[:DH, :],
                                                 rb[:])

                with ExitStack() as octx:
                    ps_out = octx.enter_context(tc.tile_pool(name="ps_caout", bufs=4, space="PSUM"))

                    def ca_outproj_half(n):
                        nsl = slice(n * 512, (n + 1) * 512)
                        for m in range(CT):
                            msl = slice(m * P, (m + 1) * P)
                            pst = ps_out.tile([P, 512], F32, tag="poca", name="poca")
                            for hp in range(NH // 2):
                                nc.tensor.matmul(pst[:],
                                                 lhsT=woca_sb[:, 2 * hp:2 * hp + 2, msl],
                                                 rhs=ohca[:, 2 * hp:2 * hp + 2, nsl],
                                                 perf_mode=DR,
                                                 start=(hp == 0), stop=(hp == NH // 2 - 1))
                            nc.vector.scalar_tensor_tensor(
                                x3[m][:, nsl], pst[:], 1.0 / W8SCALE,
                                x2[m][:, nsl], ALU.mult, ALU.add)

                    for n in range(NHALF):
                        ca_outproj_half(n)

            if stages < 4:
                with ExitStack() as ectx:
                    _early_out(ectx, x3)
                continue

            # ================= Phase 4: LN3 + GeGLU FFN (+ conv out) =================
            with ExitStack() as ctx:
                x4pool = ctx.enter_context(tc.tile_pool(name="x4p", bufs=1))
                x4big = x4pool.tile([P, CT, HW], F32R, tag="x4", name="x4big")
                x4 = [x4big[:, m] for m in range(CT)]
                # prefetch FFN / conv-out weights and the long residual before LN3
                w1pool = ctx.enter_context(tc.tile_pool(name="w1", bufs=8))
                w2pool = ctx.enter_context(tc.tile_pool(name="w2", bufs=1))
                xo2pool = ctx.enter_context(tc.tile_pool(name="xo2p", bufs=1))
                lin1_ap = lin1_w.ap().rearrange("(t p) n -> p t n", p=P)
                w2_sb = w2pool.tile([P, FT // 2, 2, C], F8, tag="w2t")
                nc.sync.dma_start(w2_sb[:], lin2_w.ap().rearrange(
                    "(i j p) n -> p i j n", p=P, j=2))
                w1tiles = {}
                for c in range(4):
                    wa = w1pool.tile([P, CT, C], F8, tag="w1t", name="w1a")
                    nc.sync.dma_start(wa[:], lin1_ap[:, :, c * C:(c + 1) * C])
                    wg = w1pool.tile([P, CT, C], F8, tag="w1t", name="w1g")
                    nc.sync.dma_start(wg[:], lin1_ap[:, :, FFH + c * C:FFH + (c + 1) * C])
                    w1tiles[c] = (wa, wg)
                xo2 = [xo2pool.tile([P, HW], F32, tag=f"xo2_{k}", name=f"xo2_{k}")
                       for k in range(CT)]
                xo2b = [xo2pool.tile([P, HW], F32, tag=f"xo2b_{k}", name=f"xo2b_{k}")
                        for k in range(CT)]
                for k in range(CT):
                    nc.sync.dma_start(xo2[k][:], xt_d.ap()[k * P:(k + 1) * P, :])
                    # on ACT: DVE is the loaded engine, ACT idles in this tail
                    nc.scalar.activation(xo2b[k][:], xo2[k][:], AF.Identity,
                                         bias=vt["co_b"][:, k:k + 1])
                co_sb = load_w(ctx, co_wT, CT, C, "co")

                t3, t3big = layer_norm(top, x3, "ln3", epsln, big_out=F8)

                # fp8 scaling: W1,W2 host-scaled by 32. a_sb = aps/8 + 4*b_a
                # (= 4*a_true), g_sb = gelu(gps/32 + b_g) (true), gi = 4*t_true,
                # l2ps = 32*W2 @ 4*t = 128*(t@W2) -> x4 = l2ps/128 + x3
                # (lin2_b folds into co_b on host).
                with ExitStack() as fctx:
                    gpool = fctx.enter_context(tc.tile_pool(name="geglu", bufs=3))
                    apool = fctx.enter_context(tc.tile_pool(name="a_tmp", bufs=3))
                    ps_f = fctx.enter_context(tc.tile_pool(name="ps_ffn", bufs=3, space="PSUM"))
                    ps_l2 = fctx.enter_context(tc.tile_pool(name="ps_l2", bufs=5, space="PSUM"))

                    for n in range(NHALF):
                        nsl = slice(n * 512, (n + 1) * 512)
                        l2ps = [ps_l2.tile([P, 512], F32, tag="l2ps", name=f"l2ps{m}")
                                for m in range(CT)]
                        gpair = None
                        for c in range(4):  # 640-col weight chunks
                            wa, wg = w1tiles[c]
                            for j in range(CT):  # 5 gate tiles per chunk
                                i = c * CT + j
                                jsl = slice(j * P, (j + 1) * P)
                                aps = ps_f.tile([P, 512], F32, tag="fps", name="aps")
                                gps = ps_f.tile([P, 512], F32, tag="fps", name="gps")
                                for (w_, ps_) in ((wa, aps), (wg, gps)):
                                    nc.tensor.matmul(ps_[:], lhsT=w_[:, 0:2, jsl],
                                                     rhs=t3big[:, 0:2, nsl], perf_mode=DR,
                                                     start=True, stop=False)
                                    nc.tensor.matmul(ps_[:], lhsT=w_[:, 2:4, jsl],
                                                     rhs=t3big[:, 2:4, nsl], perf_mode=DR,
                                                     start=False, stop=False)
                                    nc.tensor.matmul(ps_[:], lhsT=w_[:, 4, jsl],
                                                     rhs=t3big[:, 4, nsl],
                                                     start=False, stop=True)
                                a_sb = apool.tile([P, 512], F32R, tag="a", name="a_sb")
                                if i % 2 == 0:
                                    nc.scalar.activation(a_sb[:], aps[:], AF.Identity,
                                                         bias=lin1_b_sb[:, i:i + 1],
                                                         scale=4.0 / W8SCALE)
                                else:
                                    nc.vector.tensor_scalar(a_sb[:], aps[:],
                                                            4.0 / W8SCALE,
                                                            lin1_b_sb[:, i:i + 1],
                                                            ALU.mult, ALU.add)
                                g_sb = apool.tile([P, 512], F32R, tag="gg", name="g_sb")
                                nc.scalar.activation(g_sb[:], gps[:], AF.Gelu,
                                                     bias=lin1_b_sb[:, FT + i:FT + i + 1],
                                                     scale=1.0 / W8SCALE)
                                if i % 2 == 0:
                                    gpair = gpool.tile([P, 2, 512], F8, tag="g", name="gi")
                                eng = nc.gpsimd if i % 2 == 0 else nc.vector
                                eng.tensor_mul(gpair[:, i % 2], a_sb[:], g_sb[:])
                                if i % 2 == 1:
                                    for m in range(CT):
                                        nc.tensor.matmul(l2ps[m][:],
                                                         lhsT=w2_sb[:, i // 2, :, m * P:(m + 1) * P],
                                                         rhs=gpair[:], perf_mode=DR,
                                                         start=(i == 1), stop=(i == FT - 1))
                        for m in range(CT):
                            nc.vector.scalar_tensor_tensor(
                                x4[m][:, nsl], l2ps[m][:], 1.0 / (4.0 * W8SCALE),
                                x3[m][:, nsl], ALU.mult, ALU.add)

                if stages < 5:
                    with ExitStack() as ectx:
                        _early_out(ectx, x4)
                    continue

                # ---- conv out + long residual ----
                opool = ctx.enter_context(tc.tile_pool(name="outp", bufs=3))

                def co_consumer(m, n, pst):
                    nsl = slice(n * 512, (n + 1) * 512)
                    o = opool.tile([P, 512], F32, tag="osb", name="osb")
                    nc.vector.scalar_tensor_tensor(o[:], pst[:], 1.0,
                                                   xo2b[m][:, nsl], ALU.mult, ALU.add)
                    nc.sync.dma_start(y_d.ap()[m * P:(m + 1) * P, nsl], o[:])
                linear_cm(ctx, co_sb, x4, C, "co", co_consumer)

    nc.compile()
    return nc


def _get_program():
    if "nc" not in _CACHE:
        _CACHE["nc"] = _build()
    return _CACHE["nc"]


def _make_runner(nc, n_cores=8):
    import jax
    import numpy as _np
    from jax.experimental.shard_map import shard_map
    from jax.sharding import Mesh, PartitionSpec, NamedSharding
    from concourse import bass2jax
    import concourse.mybir as _mybir

    bass2jax.install_neuronx_cc_hook()
    partition_name = nc.partition_id_tensor.name if nc.partition_id_tensor else None

    in_names, out_names, out_avals, zero_outs = [], [], [], []
    in_dtypes = []
    for alloc in nc.m.functions[0].allocations:
        if not isinstance(alloc, _mybir.MemoryLocationSet):
            continue
        name = alloc.memorylocations[0].name
        if alloc.kind == "ExternalInput":
            if name != partition_name:
                in_names.append(name)
                in_dtypes.append(_mybir.dt.np(alloc.dtype))
        elif alloc.kind == "ExternalOutput":
            shape = tuple(alloc.tensor_shape)
            dtype = _mybir.dt.np(alloc.dtype)
            out_names.append(name)
            out_avals.append(jax.core.ShapedArray(shape, dtype))
            zero_outs.append(_np.zeros(shape, dtype))
    n_params = len(in_names)
    n_outs = len(out_avals)
    all_in_names = list(in_names) + list(out_names)
    if partition_name is not None:
        all_in_names.append(partition_name)

    def _body(*args):
        operands = list(args)
        if partition_name is not None:
            operands.append(bass2jax.partition_id_tensor())
        outs = bass2jax._bass_exec_p.bind(
            *operands,
            out_avals=tuple(out_avals),
            in_names=tuple(all_in_names),
            out_names=tuple(out_names),
            lowering_input_output_aliases=(),
            sim_require_finite=True,
            sim_require_nnan=True,
            nc=nc,
        )
        return tuple(outs)

    devices = jax.devices()[:n_cores]
    mesh = Mesh(_np.asarray(devices), ("core",))
    in_specs = (PartitionSpec("core"),) * (n_params + n_outs)
    out_specs = (PartitionSpec("core"),) * n_outs
    sharded = jax.jit(
        shard_map(_body, mesh=mesh, in_specs=in_specs, out_specs=out_specs,
                  check_rep=False),
        keep_unused=True)
    shard = NamedSharding(mesh, PartitionSpec("core"))

    def prepare(in_maps):
        per_core = [[_np.asarray(m[name]).astype(in_dtypes[i], copy=False)
                     for i, name in enumerate(in_names)] for m in in_maps]
        concat_in = [_np.concatenate([per_core[c][i] for c in range(n_cores)], axis=0)
                     for i in range(n_params)]
        concat_zeros = [_np.zeros((n_cores * z.shape[0], *z.shape[1:]), z.dtype)
                        for z in zero_outs]
        dev = [jax.device_put(a, shard) for a in concat_in + concat_zeros]
        jax.block_until_ready(dev)
        return dev

    def execute(dev_args, block=True):
        out_arrs = sharded(*dev_args)
        if block:
            jax.block_until_ready(out_arrs)
        return out_arrs

    def run(in_maps, want_outputs=True):
        out_arrs = execute(prepare(in_maps))
        if not want_outputs:
            return None
        return [
            {name: _np.asarray(out_arrs[i]).reshape(n_cores, *out_avals[i].shape)[c]
             for i, name in enumerate(out_names)}
            for c in range(n_cores)
        ]

    run.in_names = in_names
    run.prepare = prepare
    run.execute = execute
    return run


def _get_runner():
    if "runner" not in _CACHE:
        _CACHE["runner"] = _make_runner(_get_program())
    return _CACHE["runner"]


def _vp_init():
    v = np.zeros((P, NH, NH, NHP), np.float32)
    v[:, :, :, 96] = 1.0
    return v.reshape(P, NH * NH * NHP)


def _oh_row():
    v = np.zeros((1, NH, HW), np.float32)
    v[0, 0, :] = 1.0
    return v.reshape(1, NH * HW)


def _vca_init():
    v = np.zeros((LCTXP, NH, 97), np.float32)
    v[:LCTX, :, 96] = 1.0
    return v.reshape(LCTXP, NH * 97)


def _pack_out_w(w, b):
    """[C, C] out-proj weight + bias -> [DHB*NH, C] with bias on row 80 of
    head 0 (matching the oh bias-carrier row)."""
    out = np.zeros((DHB * NH, w.shape[1]), np.float32)
    for h in range(NH):
        out[h * DHB:h * DHB + DH] = w[h * DH:(h + 1) * DH]
    out[DH] = b
    return out


def _make_in_maps(inputs):
    x = np.asarray(inputs["x"], dtype=np.float32)
    context = np.asarray(inputs["context"], dtype=np.float32)
    B = x.shape[0]

    G = np.zeros((C, GROUPS), np.float32)
    for c in range(C):
        G[c, c // GSIZE] = 1.0

    f32 = lambda name: np.asarray(inputs[name], np.float32)
    # fold LN scales into downstream weights; LN biases become projection
    # biases (q/k per-head partition biases; v-bias folds into out_b because
    # attention weights sum to 1).
    sa_in = f32("sa_in_w") * f32("ln1_s")[:, None]
    sa_in[:, :C] *= SCALE
    qkvb = f32("ln1_b") @ (f32("sa_in_w"))
    qkvb[:C] *= SCALE
    qkb = np.zeros((DH, 2 * NH), np.float32)
    for h in range(NH):
        qkb[:, h] = qkvb[h * DH:(h + 1) * DH]
        qkb[:, NH + h] = qkvb[C + h * DH:C + (h + 1) * DH]
    sa_out_b = f32("sa_out_b") + qkvb[2 * C:] @ f32("sa_out_w")

    ca_q = f32("ca_q_w") * f32("ln2_s")[:, None] * SCALE
    caqv = (f32("ln2_b") @ f32("ca_q_w")) * SCALE
    caqb = np.zeros((DH, NH), np.float32)
    for h in range(NH):
        caqb[:, h] = caqv[h * DH:(h + 1) * DH]

    lin1 = f32("lin1_w") * f32("ln3_s")[:, None] * W8SCALE
    lin1_b = f32("lin1_b") + f32("ln3_b") @ f32("lin1_w")
    lin1_b[:FFH] *= 4.0  # a_sb carries 4*a_true (see device-side comment)
    co_b = f32("co_b") + f32("lin2_b") @ f32("co_w")

    shared = {
        "conv1_wT": np.ascontiguousarray(f32("conv1_w").T),
        "co_wT": np.ascontiguousarray(f32("co_w").T),
        "G": G, "G2": np.ascontiguousarray(G.T),
        "recipC": np.full((P, 1), 1.0 / C, np.float32),
        "vpinit": _vp_init(),
        "ohrow": _oh_row(),
        "vcainit": _vca_init(),
        "sa_in_w": np.ascontiguousarray(sa_in * W8SCALE),
        "sa_out_w": np.ascontiguousarray(
            _pack_out_w(f32("sa_out_w"), sa_out_b) * W8SCALE),
        "ca_q_w": np.ascontiguousarray(ca_q * W8SCALE),
        "ca_out_w": np.ascontiguousarray(
            _pack_out_w(f32("ca_out_w"), f32("ca_out_b")) * W8SCALE),
        "qkb": qkb, "caqb": caqb,
        "lin1_w": np.ascontiguousarray(lin1),
        "lin1_b": np.ascontiguousarray(lin1_b),
        "lin2_w": np.ascontiguousarray(f32("lin2_w") * W8SCALE),
        "co_b": np.ascontiguousarray(co_b),
    }
    for name in ["ca_k_w", "ca_v_w", "gn_s", "gn_b", "conv1_b"]:
        shared[name] = np.ascontiguousarray(f32(name))

    in_maps = []
    for b in range(B):
        m = dict(shared)
        m["xt"] = np.ascontiguousarray(x[b].reshape(C, HW))
        m["xtb"] = m["xt"]
        ct = np.zeros((DCTX, LCTXP), np.float32)
        ct[:, :LCTX] = context[b].T
        m["ctxT"] = ct
        in_maps.append(m)
    return in_maps


def kernel(**inputs) -> np.ndarray:
    run = _get_runner()
    in_maps = _make_in_maps(inputs)
    results = run(in_maps)
    out = np.stack([results[b]["y"] for b in range(8)], axis=0)
    return out.reshape(8, C, 32, 32).astype(np.float32)

